# revision 1
# baseline (speedup 1.0000x reference)
"""CrossAttention kernel for 8x TRN2 NeuronCores (Bass/Tile).

Reference computation (per batch b of 16, heads h=8, n=1024, d_model=512, dh=64):
    q = x @ Wq.T, k = x @ Wk.T, v = x @ Wv.T          (per-head slices)
    sim = q k^T * scale + rel_bias[h]
    attn = softmax(sim, axis=-1)
    out = (attn @ v) re-assembled over heads, then @ Wo.T + bo

Sharding: data-parallel over batch, 2 batches per core x 8 cores.

Per-core device algorithm (all matmuls run as float32r = full-rate fp32-ish):
  - host supplies x^T per batch ([512, 1024]) and pre-transposed weights, so
    every matmul's contraction dim lands on SBUF partitions with no on-device
    transposes.
  - sim is computed TRANSPOSED: simT[j, i] = (K Q^T)[j, i], j on partitions.
    Softmax max-subtraction is skipped (logits are O(1) by construction);
    exp runs on ACT, the rel_bias add is folded as exp(sim)*exp(bias) with
    exp(bias^T) precomputed on host (bf16), multiplied on DVE in bf16 (2x mode).
  - attn @ V becomes OT[d, i] = sum_j Vaug[j, d|1] * et[j, i] on the PE with V
    in its NATURAL layout as lhsT; an appended ones-column yields the softmax
    denominator l[i] in the same matmul. Normalization multiplies by 1/l
    broadcast across partitions via a K=1 matmul.
  - final projection consumes attn_out^T directly as lhsT; bo is added via a
    K=1 ones matmul into the same PSUM accumulation group.
"""

import numpy as np
import ml_dtypes

HEADS = 8
DH = 64
B = 16
N = 1024
D = 512  # d_model = inner
SCALE = DH ** -0.5
NCORES = 8
BPC = B // NCORES  # batches per core

F32 = None  # set lazily (mybir import)


def build_nc(n=N, bpc=BPC, ps_a_bufs=3, ot_bufs=4, eb_bufs=6, eqk_bufs=6, et_bufs=15, xt_bufs=None, phases="PAF", debug=False):
    import concourse.mybir as mybir
    import concourse.tile as tile
    from concourse import bacc

    f32 = mybir.dt.float32
    f32r = mybir.dt.float32r
    bf16 = mybir.dt.bfloat16
    Exp = mybir.ActivationFunctionType.Exp

    NT = n // 128            # n tiles of 128
    NIH = max(1, n // 512)   # i-halves
    IW = min(512, n)         # i-slice width (psum free dim)
    KP = D // 128            # d_model k-tiles (4)
    HP = HEADS // 2          # head pairs

    nc = bacc.Bacc(None, target_bir_lowering=False)

    xT_d = nc.dram_tensor("xT", [bpc, D, n], f32r, kind="ExternalInput")
    wq_d = nc.dram_tensor("WqT", [D, D], f32r, kind="ExternalInput")   # pre-scaled
    wk_d = nc.dram_tensor("WkT", [D, D], f32r, kind="ExternalInput")
    wv_d = nc.dram_tensor("WvT", [D, D], f32r, kind="ExternalInput")
    wo_d = nc.dram_tensor("WoT", [D, D], f32r, kind="ExternalInput")
    eb_d = nc.dram_tensor("expBT", [HEADS, n, n], bf16, kind="ExternalInput")
    bo_d = nc.dram_tensor("bo", [1, D], f32r, kind="ExternalInput")
    ones_d = nc.dram_tensor("ones", [1, 128], f32r, kind="ExternalInput")
    out_d = nc.dram_tensor("out", [bpc, n, D], f32, kind="ExternalOutput")
    if debug:
        dqt_d = nc.dram_tensor("dQT", [128, n], f32, kind="ExternalOutput")
        dkt_d = nc.dram_tensor("dKT", [128, n], f32, kind="ExternalOutput")
        dva_d = nc.dram_tensor("dVA", [128, HEADS * (DH + 1)], mybir.dt.bfloat16, kind="ExternalOutput")
        det_d = nc.dram_tensor("dET", [128, n], mybir.dt.bfloat16, kind="ExternalOutput")
        dot_d = nc.dram_tensor("dOT", [DH + 1, 512], f32, kind="ExternalOutput")
        dao_d = nc.dram_tensor("dAO", [128, n], f32, kind="ExternalOutput")

    with tile.TileContext(nc) as tc:
        with (
            tc.tile_pool(name="pers", bufs=1) as pers,       # persistent tiles
            tc.tile_pool(name="osb", bufs=4) as osbp,
            tc.tile_pool(name="lr", bufs=2) as lrp,
        ):
            # ---- persistent tiles
            QT = [[pers.tile([128, n], f32r, tag=f"qt{bi}_{ip}", name=f"qt{bi}_{ip}") for ip in range(KP)]
                  for bi in range(bpc)]
            KT = [[pers.tile([128, n], f32r, tag=f"kt{bi}_{ip}", name=f"kt{bi}_{ip}") for ip in range(KP)]
                  for bi in range(bpc)]
            VA = [[pers.tile([128, HEADS * (DH + 1)], bf16, tag=f"va{bi}_{nt}", name=f"va{bi}_{nt}")
                   for nt in range(NT)] for bi in range(bpc)]
            AO = [[pers.tile([128, n], f32r, tag=f"ao{bi}_{kp}", name=f"ao{bi}_{kp}") for kp in range(KP)]
                  for bi in range(bpc)]
            wo_s = [pers.tile([128, D], f32r, tag=f"wo{kp}", name=f"wo{kp}") for kp in range(KP)]
            bo_s = pers.tile([1, D], f32r, tag="bo")
            ones128 = pers.tile([1, 128], f32r, tag="ones128")
            nc.sync.dma_start(out=ones128[:], in_=ones_d[:])
            nc.sync.dma_start(out=bo_s[:], in_=bo_d[:])
            for kp in range(KP):
                nc.sync.dma_start(out=wo_s[kp][:], in_=wo_d[kp * 128:(kp + 1) * 128, :])

            wqkv = tc.alloc_tile_pool(name="wqkv", bufs=1)
            xtp = tc.alloc_tile_pool(name="xt", bufs=xt_bufs or (KP + 1))
            ps_p = tc.alloc_tile_pool(name="ps_p", bufs=4, space="PSUM")
            wq_s = [wqkv.tile([128, D], f32r, tag=f"wq{kp}", name=f"wq{kp}") for kp in range(KP)]
            wk_s = [wqkv.tile([128, D], f32r, tag=f"wk{kp}", name=f"wk{kp}") for kp in range(KP)]
            wv_s = [wqkv.tile([128, D], f32r, tag=f"wv{kp}", name=f"wv{kp}") for kp in range(KP)]
            for kp in range(KP):
                nc.sync.dma_start(out=wq_s[kp][:], in_=wq_d[kp * 128:(kp + 1) * 128, :])
                nc.sync.dma_start(out=wk_s[kp][:], in_=wk_d[kp * 128:(kp + 1) * 128, :])
                nc.sync.dma_start(out=wv_s[kp][:], in_=wv_d[kp * 128:(kp + 1) * 128, :])

            # ================= Phase P: projections =================
            for bi in (range(bpc) if "P" in phases else ()):
                xt = [xtp.tile([128, n], f32r, tag="xt", name="xt") for _ in range(KP)]
                for kp in range(KP):
                    nc.sync.dma_start(out=xt[kp][:], in_=xT_d[bi, kp * 128:(kp + 1) * 128, :])
                # QT/KT: [inner, n] = W^T.T @ x^T
                for W_s, DST in ((wq_s, QT[bi]), (wk_s, KT[bi])):
                    for ip in range(KP):
                        for nh in range(NIH):
                            pt = ps_p.tile([128, IW], f32, tag="mm")
                            for kp in range(KP):
                                nc.tensor.matmul(
                                    pt[:],
                                    W_s[kp][:, ip * 128:(ip + 1) * 128],
                                    xt[kp][:, nh * IW:(nh + 1) * IW],
                                    start=(kp == 0), stop=(kp == KP - 1),
                                )
                            nc.vector.tensor_copy(
                                out=DST[ip][:, nh * IW:(nh + 1) * IW], in_=pt[:])
                # V natural [n, inner] -> VA bf16 with ones cols
                for nt in range(NT):
                    pt = ps_p.tile([128, D], f32, tag="mm")
                    for kp in range(KP):
                        nc.tensor.matmul(
                            pt[:],
                            xt[kp][:, nt * 128:(nt + 1) * 128],
                            wv_s[kp][:],
                            start=(kp == 0), stop=(kp == KP - 1),
                        )
                    va = VA[bi][nt]
                    nc.gpsimd.memset(va[:], 1.0)
                    dst3 = va[:].rearrange("p (h c) -> p h c", c=DH + 1)[:, :, 0:DH]
                    src3 = pt[:].rearrange("p (h c) -> p h c", c=DH)
                    nc.vector.tensor_copy(out=dst3, in_=src3)

            ps_p.release()
            xtp.release()
            wqkv.release()
            ebp = tc.alloc_tile_pool(name="eb", bufs=eb_bufs)
            eqkp = tc.alloc_tile_pool(name="eqk", bufs=eqk_bufs)
            etp = tc.alloc_tile_pool(name="et", bufs=et_bufs)
            ps_sim = tc.alloc_tile_pool(name="ps_sim", bufs=2, space="PSUM")
            ps_ot = tc.alloc_tile_pool(name="ps_ot", bufs=ot_bufs, space="PSUM")

            # ================= Phase A: attention =================
            for hp in (range(HP) if "A" in phases else ()):
                h0, h1 = 2 * hp, 2 * hp + 1
                streams = [(h, bi) for bi in range(bpc) for h in (h0, h1)]
                et_t = {}
                for jt in range(NT):
                    eb = {}
                    for h in (h0, h1):
                        t = ebp.tile([128, n], bf16, tag="eb", name="eb")
                        nc.sync.dma_start(out=t[:], in_=eb_d[h, jt * 128:(jt + 1) * 128, :])
                        eb[h] = t
                    for (h, bi) in streams:
                        po = (h % 2) * 64
                        ktile = KT[bi][h // 2]
                        qtile = QT[bi][h // 2]
                        sp = ps_sim.tile([128, n], f32, tag="sim", name="sim")
                        for ihh in range(NIH):
                            nc.tensor.matmul(
                                sp[:, ihh * IW:(ihh + 1) * IW],
                                ktile[po:po + 64, jt * 128:(jt + 1) * 128],
                                qtile[po:po + 64, ihh * IW:(ihh + 1) * IW],
                                start=True, stop=True,
                            )
                        eq = eqkp.tile([128, n], bf16, tag="eqk")
                        nc.scalar.activation(eq[:], sp[:], Exp)
                        et = etp.tile([128, n], bf16, tag="et")
                        nc.vector.tensor_mul(out=et[:], in0=eq[:], in1=eb[h][:])
                        et_t[(h, bi, jt)] = et
                        if debug and h == 0 and bi == 0 and jt == 0:
                            nc.sync.dma_start(out=det_d[:], in_=et[:])
                for (h, bi) in streams:
                    po = (h % 2) * 64
                    for ihh in range(NIH):
                        o = ps_ot.tile([DH + 1, IW], f32, tag="ot", name="ot")
                        for jt in range(NT):
                            nc.tensor.matmul(
                                o[:],
                                VA[bi][jt][:, h * (DH + 1):(h + 1) * (DH + 1)],
                                et_t[(h, bi, jt)][:, ihh * IW:(ihh + 1) * IW],
                                start=(jt == 0), stop=(jt == NT - 1),
                            )
                        if debug and h == 0 and bi == 0 and ihh == 0:
                            ots = osbp.tile([DH + 1, IW], f32, tag="dots", name="dots", bufs=1)
                            nc.vector.tensor_copy(out=ots[:], in_=o[:])
                            nc.sync.dma_start(out=dot_d[:], in_=ots[:])
                        lcp = lrp.tile([DH + 1, IW], f32, tag="lcp", name="lcp")
                        nc.vector.tensor_copy(out=lcp[DH:DH + 1, :], in_=o[DH:DH + 1, :])
                        lr0 = lrp.tile([1, IW], f32, tag="lr0", name="lr0")
                        nc.sync.dma_start(out=lr0[:], in_=lcp[DH:DH + 1, :])
                        lrr = lrp.tile([1, IW], f32, tag="lrr", name="lrr")
                        nc.vector.reciprocal_approx_fast(out=lrr[:], in_=lr0[:])
                        lb = osbp.tile([DH, IW], f32, tag="lb", name="lb", bufs=2)
                        nc.gpsimd.partition_broadcast(lb[:], lrr[:], channels=DH)
                        if po == 0:
                            nc.vector.tensor_mul(
                                out=AO[bi][h // 2][0:DH, ihh * IW:(ihh + 1) * IW],
                                in0=o[0:DH, :], in1=lb[:])
                        else:
                            tmpo = osbp.tile([DH, IW], f32r, tag="tmpo", name="tmpo", bufs=2)
                            nc.vector.tensor_mul(out=tmpo[:], in0=o[0:DH, :], in1=lb[:])
                            nc.sync.dma_start(
                                out=AO[bi][h // 2][po:po + DH, ihh * IW:(ihh + 1) * IW],
                                in_=tmpo[:])

            ps_ot.release()
            ps_sim.release()
            etp.release()
            eqkp.release()
            ebp.release()
            ps_f = tc.alloc_tile_pool(name="ps_f", bufs=4, space="PSUM")

            # ================= Phase F: output projection =================
            for bi in (range(bpc) if "F" in phases else ()):
                for nt in range(NT):
                    fp = ps_f.tile([128, D], f32, tag="mm")
                    for kp in range(KP):
                        nc.tensor.matmul(
                            fp[:],
                            AO[bi][kp][:, nt * 128:(nt + 1) * 128],
                            wo_s[kp][:],
                            start=(kp == 0), stop=False,
                        )
                    nc.tensor.matmul(fp[:], ones128[:], bo_s[:],
                                     start=False, stop=True)
                    fo = osbp.tile([128, D], f32, tag="fo", name="fo")
                    nc.vector.tensor_copy(out=fo[:], in_=fp[:])
                    nc.sync.dma_start(out=out_d[bi, nt * 128:(nt + 1) * 128, :], in_=fo[:])
            ps_f.release()
            if debug:
                nc.sync.dma_start(out=dqt_d[:], in_=QT[0][0][:].bitcast(f32))
                nc.sync.dma_start(out=dkt_d[:], in_=KT[0][0][:].bitcast(f32))
                nc.sync.dma_start(out=dva_d[:], in_=VA[0][0][:])
                nc.sync.dma_start(out=dao_d[:], in_=AO[0][0][:].bitcast(f32))

    nc.compile()
    return nc


def prep_inputs(x, Wq, Wk, Wv, rel_bias, Wo, bo, n=N, bpc=BPC, ncores=NCORES):
    """Host-side sharding/layout prep. Returns in_maps (one dict per core)."""
    x = np.ascontiguousarray(x, dtype=np.float32)
    xT = np.ascontiguousarray(x.transpose(0, 2, 1))        # [B, D, n]
    WqT = np.ascontiguousarray(Wq.T * np.float32(SCALE), dtype=np.float32)
    WkT = np.ascontiguousarray(Wk.T, dtype=np.float32)
    WvT = np.ascontiguousarray(Wv.T, dtype=np.float32)
    WoT = np.ascontiguousarray(Wo.T, dtype=np.float32)
    expBT = np.ascontiguousarray(
        np.exp(rel_bias.astype(np.float32).transpose(0, 2, 1))
    ).astype(ml_dtypes.bfloat16)                            # [H, n(j), n(i)]
    bo2 = np.ascontiguousarray(bo, dtype=np.float32).reshape(1, D)
    in_maps = []
    for c in range(ncores):
        in_maps.append({
            "xT": np.ascontiguousarray(xT[c * bpc:(c + 1) * bpc]),
            "WqT": WqT, "WkT": WkT, "WvT": WvT, "WoT": WoT,
            "expBT": expBT, "bo": bo2, "ones": np.ones((1, 128), np.float32),
        })
    return in_maps


_CACHE = {}


def kernel(x, Wq, Wk, Wv, rel_bias, Wo, bo):
    from concourse.bass_utils import run_bass_kernel_spmd

    if "nc" not in _CACHE:
        _CACHE["nc"] = build_nc()
    nc = _CACHE["nc"]
    in_maps = prep_inputs(x, Wq, Wk, Wv, rel_bias, Wo, bo)
    res = run_bass_kernel_spmd(nc, in_maps, core_ids=list(range(NCORES)))
    out = np.concatenate([res.results[c]["out"] for c in range(NCORES)], axis=0)
    return np.ascontiguousarray(out, dtype=np.float32)

